# revision 17
# baseline (speedup 1.0000x reference)
"""Trainium2 Bass kernel: batched locally-weighted ridge regression.

Per test point t: K[t,n] = exp(-|xte_t - xtr_n|^2 / (2 ls^2));
  A_t = Xtild^T diag(K[t]) Xtild + REG*I ; b_t = Xtild^T (K[t] * Y)
  ypred_t = xtild_t . A_t^{-1} b_t
Sharding: data-parallel over the 4096 test points -> 8 cores x 512.

On-device math uses a scaled kernel K' = exp((S - sn/2) * c2)
(c2 = 1/ls^2); the dropped exp(-st*c2/2) per-test factor rescales A_t and
b_t identically, so beta is preserved by using a per-test ridge
REG_t = REG * exp(st*c2/2) (host-computed).  The -sn/2 bias rides along
as an extra contraction row of the host-packed [XtrT | XteT] operand, so
the gram matmul emits S - sn/2 directly and the exps need no bias.

ypred is computed via a BORDERED elimination: per system the 33x33 matrix
  M = [[A, b], [xtild_t^T, 0]]
is reduced by 32 steps of Gaussian elimination (no pivoting; A is SPD),
after which M[32,32] = -xtild_t^T A^{-1} b = -ypred_t.  This removes the
back-substitution and prediction dot-product phases entirely.

Engine split per core:
  PE  : gram (S - sn/2) and XWX/XWy matmuls, all in fp32r (1 cycle/row at
        free size >= 256); the four XWX PSUM accumulations of a t-half
        run in one interleaved chunk loop trailing the exps
  ACT : exps, PSUM evacuations, border-row init, output negation
  DVE : Z-build part, left-column elimination updates (multiplier column
        via divide)
  Pool: Z-build part, right-column elimination updates (gpsimd
        tensor_tensor runs at 0.83 ns/elem vs DVE 1.04)

The elimination runs as two 2-t-tile chains; each keeps a parity-double-
buffered multiplier column so Pool lags DVE freely without
write-after-read stalls.
"""

import numpy as np

import concourse.bacc as bacc
import concourse.bass as bass
import concourse.mybir as mybir
from concourse.bass import ds, ts
from concourse.bass_utils import run_bass_kernel_spmd
from concourse.tile import TileContext

F32 = mybir.dt.float32
F32R = mybir.dt.float32r
P = 128
N_TRAIN = 2048
D = 31
DP = 32          # 1 + D
DB = 33          # bordered system size (DP rows of A + border row)
N_TEST = 4096
NCORES = 8
TS = N_TEST // NCORES   # 512 test points per core
NT = TS // P            # 4 t-tiles
NK = N_TRAIN // P       # 16 train chunks
REG = 1e-6
H = 16
NZ = DP * H + H * H + DP             # 512 + 256 + 32 = 800
WLF = 0.55           # fraction of elimination columns updated on DVE
NMISC = NK * D + NT * D + NK + NT    # xtr | xte | ytr | regt columns


def _build_nc(c2: float):
    """Build the single-core Bass program (SPMD across 8 cores)."""
    nc = bacc.Bacc(trn_type="TRN2")

    xm_d = nc.dram_tensor("xmisc", [P, NMISC], F32, kind="ExternalInput")
    xT_d = nc.dram_tensor("xT", [DP, N_TRAIN + TS], F32R,
                          kind="ExternalInput")
    out_d = nc.dram_tensor("ypred", [TS, 1], F32, kind="ExternalOutput")

    HTC = (N_TRAIN + TS) // 2

    with TileContext(nc) as tc:
        with (
            tc.tile_pool(name="sb", bufs=1) as sb,
            tc.tile_pool(name="pgram", bufs=4, space="PSUM") as pgram,
            tc.tile_pool(name="pxwx", bufs=1, space="PSUM") as pxwx,
        ):
            # ---- load inputs: 3 DMAs (packed misc, transposed feats) ----
            xm = sb.tile([P, NMISC], F32)
            nc.sync.dma_start(xm, xm_d[:, :])
            xT = sb.tile([DP, N_TRAIN + TS], F32R)
            nc.scalar.dma_start(xT[:, 0:1024], xT_d[:, 0:1024])
            nc.gpsimd.dma_start(xT[:, 1024:2048], xT_d[:, 1024:2048])
            nc.sync.dma_start(xT[:, 2048:], xT_d[:, 2048:])
            xtr = xm[:, 0:NK * D].rearrange("p (c d) -> p c d", c=NK)
            xte = xm[:, NK * D:NK * D + NT * D].rearrange(
                "p (t d) -> p t d", t=NT)
            ytr = xm[:, NK * D + NT * D:NK * D + NT * D + NK]
            regt = xm[:, NK * D + NT * D + NK:]

            # ---- Xtild chunks [128, NK, 32] (ones column + Xtrain) ----
            xt = sb.tile([P, NK, DP], F32)
            nc.vector.memset(xt[:, :, 0:1], 1.0)
            nc.scalar.copy(xt[:, :, 1:DP], xtr)

            # ---- Z = [xtild_d * xtild_e (768 unique) | xtild * y (32)] ----
            # cols 0:512   : (d, e) for d in 0..31, e in 16..31   (Pool)
            # cols 512:768 : (d, e) for d, e in 0..15             (DVE)
            # cols 768:800 : xtild * y                            (DVE)
            zz = sb.tile([P, NK, NZ], F32R)
            zzv = zz[:, :, 0:DP * H].rearrange("p k (d e) -> p k d e", d=DP)
            nc.vector.tensor_mul(
                zz[:, :, DP * H:DP * H + H * H].rearrange(
                    "p k (d e) -> p k d e", d=H),
                xt[:, :, 0:H, None].broadcast_to([P, NK, H, H]),
                xt[:, :, None, 0:H].broadcast_to([P, NK, H, H]),
            )
            nc.vector.tensor_mul(
                zz[:, :, DP * H + H * H:], xt,
                ytr[:, :, None].broadcast_to([P, NK, DP]),
            )
            nc.gpsimd.tensor_mul(
                zzv[:, :, :, 0:8],
                xt[:, :, :, None].broadcast_to([P, NK, DP, 8]),
                xt[:, :, None, H:H + 8].broadcast_to([P, NK, DP, 8]),
            )
            nc.gpsimd.tensor_mul(
                zzv[:, :, :, 8:16],
                xt[:, :, :, None].broadcast_to([P, NK, DP, 8]),
                xt[:, :, None, H + 8:DP].broadcast_to([P, NK, DP, 8]),
            )

            # ---- ga: bordered systems [A | b ; xtild_t^T | 0] ----
            ga = sb.tile([P, NT, DB, DB], F32)
            # border row (row 32): [1, xte_t, 0] — disjoint from evacs
            nc.vector.memset(ga[:, :, DP, 0:1], 1.0)
            nc.vector.memset(ga[:, :, DP, DP:DB], 0.0)
            nc.scalar.copy(ga[:, :, DP, 1:DP], xte)
            # lower-left quadrant is never read by the symmetric
            # elimination but must be finite; zero it once (no mirror)
            nc.vector.memset(ga[:, :, H:DP, 0:H], 0.0)

            # ---- gram + exp per t-half; XWX per t-half; eliminate ----
            kp = sb.tile([P, NK, TS], F32R)
            ga_sw = ga[:].rearrange("p b r c -> p b c r")
            ga_flat = ga[:].rearrange("p b r c -> p b (r c)")
            ga_diag = ga_flat[:, :, ::DB + 1]    # [128, NT, 33] diagonal
            fbuf0 = sb.tile([P, 2, 2, DP], F32)
            fbuf1 = sb.tile([P, 2, 2, DP], F32)
            tbL = sb.tile([P, 2, DP, DP], F32)
            tbR = sb.tile([P, 2, DP, DP], F32)
            tb2 = sb.tile([P, 2, DP, DP], F32)
            tbB = sb.tile([P, 2, DP], F32)
            yp = sb.tile([P, NT], F32)
            outv = out_d.rearrange("(t p) one -> p (t one)", p=P)

            def eliminate(b0, fbuf):
                """Bordered SYMMETRIC elimination chain for t-tiles b0,b0+1.

                Factors come from the pivot row only (A symmetric), so the
                lower triangle is never read; the update covers
                  R1: rows k+1..k+hr        x cols k+1..32  (DVE|Pool split)
                  R2: rows k+hr+1..31       x cols k+hr+1..32  (Pool)
                  B : border row 32         x cols k+1..32  (Pool)
                ~3/4 of the full-square work.  Lower-triangle entries inside
                R1/R2 receive garbage-but-finite values and are never used.
                """
                bs = slice(b0, b0 + 2)
                for k in range(DP):
                    w = DP - k       # cols k+1..32
                    hr = min(max(1, w // 2), D - k)   # R1 rows k+1..k+hr
                    rs = fbuf[:, :, k % 2, :w]
                    nc.vector.tensor_tensor(
                        rs,
                        ga[:, bs, k, k + 1:DB],
                        ga[:, bs, k, k:k + 1].broadcast_to([P, 2, w]),
                        mybir.AluOpType.divide,
                    )
                    if hr > 0:
                        wL = w if w <= 2 else max(1, int(round(WLF * w)))
                        wR = w - wL
                        rowf = ga[:, bs, k, k + 1:k + 1 + hr]
                        nc.vector.tensor_mul(
                            tbL[:, :, :hr, :wL],
                            rowf[:, :, :, None].broadcast_to([P, 2, hr, wL]),
                            rs[:, :, None, :wL].broadcast_to([P, 2, hr, wL]),
                        )
                        nc.vector.tensor_sub(
                            ga[:, bs, k + 1:k + 1 + hr, k + 1:k + 1 + wL],
                            ga[:, bs, k + 1:k + 1 + hr, k + 1:k + 1 + wL],
                            tbL[:, :, :hr, :wL],
                        )
                        if wR:
                            nc.gpsimd.tensor_mul(
                                tbR[:, :, :hr, :wR],
                                rowf[:, :, :, None].broadcast_to(
                                    [P, 2, hr, wR]),
                                rs[:, :, None, wL:w].broadcast_to(
                                    [P, 2, hr, wR]),
                            )
                            nc.gpsimd.tensor_sub(
                                ga[:, bs, k + 1:k + 1 + hr, k + 1 + wL:DB],
                                ga[:, bs, k + 1:k + 1 + hr, k + 1 + wL:DB],
                                tbR[:, :, :hr, :wR],
                            )
                    r20 = k + 1 + hr          # R2 rows r20..31
                    m2 = DP - r20
                    if m2 > 0:
                        w2 = DB - r20         # cols r20..32
                        nc.gpsimd.tensor_mul(
                            tb2[:, :, :m2, :w2],
                            ga[:, bs, k, r20:DP][:, :, :, None].broadcast_to(
                                [P, 2, m2, w2]),
                            rs[:, :, None, hr:w].broadcast_to([P, 2, m2, w2]),
                        )
                        nc.gpsimd.tensor_sub(
                            ga[:, bs, r20:DP, r20:DB],
                            ga[:, bs, r20:DP, r20:DB],
                            tb2[:, :, :m2, :w2],
                        )
                    # border row
                    nc.gpsimd.tensor_mul(
                        tbB[:, :, :w],
                        ga[:, bs, DP, k][:, :, None].broadcast_to([P, 2, w]),
                        rs,
                    )
                    nc.gpsimd.tensor_sub(
                        ga[:, bs, DP, k + 1:DB],
                        ga[:, bs, DP, k + 1:DB],
                        tbB[:, :, :w],
                    )
                # ypred = -M[32, 32] for the chain's two t-tiles
                nc.scalar.mul(yp[:, bs], ga[:, bs, DP, DP], -1.0)
                nc.sync.dma_start(outv[:, bs], yp[:, bs])

            CHUNKS = ((DP * H, NZ), (0, DP * H))
            # grams + exps for ALL tiles up front, 512 wide (16 exps total)
            for c in range(NK):
                sg = pgram.tile([P, TS], F32, tag="sg")
                for h in range(2):
                    hc = slice(N_TRAIN + h * 2 * P, N_TRAIN + (h + 1) * 2 * P)
                    nc.tensor.matmul(
                        sg[:, ds(h * 2 * P, 2 * P)],
                        xT[0:DP, ts(c, P)],
                        xT[0:DP, hc],
                        start=True, stop=True,
                    )
                nc.scalar.activation(
                    kp[:, c, :], sg,
                    mybir.ActivationFunctionType.Exp,
                    scale=c2,
                )
            for h in range(2):
                t0, t1 = 2 * h, 2 * h + 1
                # all four PSUM groups accumulate in one interleaved c-loop,
                # trailing the exps chunk by chunk
                pxs = {}
                for t in (t0, t1):
                    for (c0, c1) in CHUNKS:
                        px = pxwx.tile([P, 512], F32, tag=f"px{t % 2}_{c0}")
                        pxs[(t, c0)] = px
                for c in range(NK):
                    for t in (t0, t1):
                        for (c0, c1) in CHUNKS:
                            nc.tensor.matmul(
                                pxs[(t, c0)][:, :c1 - c0],
                                kp[:, c, ts(t, P)],
                                zz[:, c, c0:c1],
                                start=(c == 0), stop=(c == NK - 1),
                            )
                for t in (t0, t1):
                    px288 = pxs[(t, DP * H)]
                    px512 = pxs[(t, 0)]
                    # top-left quadrant + rhs column
                    nc.scalar.copy(
                        ga[:, t, 0:H, 0:H],
                        px288[:, 0:H * H].rearrange("p (r c) -> p r c", r=H),
                    )
                    nc.scalar.copy(
                        ga[:, t, 0:DP, DP], px288[:, H * H:H * H + DP])
                    # cols e=16..31, all rows d
                    nc.scalar.copy(
                        ga[:, t, 0:DP, H:DP],
                        px512[:, :].rearrange("p (r c) -> p r c", r=DP),
                    )
                # ridge on the diagonal, then eliminate this half
                nc.vector.tensor_add(
                    ga_diag[:, t0:t0 + 2, 0:DP],
                    ga_diag[:, t0:t0 + 2, 0:DP],
                    regt[:, t0:t0 + 2, None].broadcast_to([P, 2, DP]),
                )
                eliminate(t0, fbuf0 if h == 0 else fbuf1)

    nc.finalize()
    return nc


_cache: dict[float, object] = {}


def _get_nc(c2: float):
    if c2 not in _cache:
        _cache[c2] = _build_nc(c2)
    return _cache[c2]


def _build_xT(Xtrain, shard, c2):
    """[XtrT | XteT] plus a contraction row carrying -sn/2 so the gram
    matmul emits S - sn/2 directly."""
    out = np.empty((DP, N_TRAIN + TS), np.float32)
    out[0:D, 0:N_TRAIN] = Xtrain.T
    out[0:D, N_TRAIN:] = shard.T
    sn = np.sum(np.float32(Xtrain) ** 2, axis=1, dtype=np.float32)
    out[D, 0:N_TRAIN] = -0.5 * sn
    out[D, N_TRAIN:] = 1.0
    return out


def _build_xmisc(Xtrain, shard, Ytrain, c2):
    """Pack xtrain chunks | xtest tiles | ytrain chunks | regt."""
    out = np.empty((P, NMISC), np.float32)
    for c in range(NK):
        out[:, c * D:(c + 1) * D] = Xtrain[c * P:(c + 1) * P]
    o = NK * D
    for t in range(NT):
        out[:, o + t * D:o + (t + 1) * D] = shard[t * P:(t + 1) * P]
    o += NT * D
    for c in range(NK):
        out[:, o + c] = Ytrain[c * P:(c + 1) * P, 0]
    o += NK
    st = np.sum(np.float64(shard) ** 2, axis=1)
    regt = np.float32(REG * np.exp(0.5 * c2 * st))
    for t in range(NT):
        out[:, o + t] = regt[t * P:(t + 1) * P]
    return out


def kernel(Ytrain, Xtrain, Xtest, log_lengthscale, _trace=False):
    Ytrain = np.ascontiguousarray(np.asarray(Ytrain, dtype=np.float32))
    Xtrain = np.ascontiguousarray(np.asarray(Xtrain, dtype=np.float32))
    Xtest = np.ascontiguousarray(np.asarray(Xtest, dtype=np.float32))
    lls = float(np.asarray(log_lengthscale, dtype=np.float32))
    c2 = float(np.exp(np.float32(-2.0 * lls)))

    nc = _get_nc(c2)
    in_maps = []
    for core in range(NCORES):
        shard = np.ascontiguousarray(Xtest[core * TS:(core + 1) * TS])
        in_maps.append({
            "xmisc": _build_xmisc(Xtrain, shard, Ytrain, c2),
            "xT": _build_xT(Xtrain, shard, c2),
        })
    res = run_bass_kernel_spmd(nc, in_maps, list(range(NCORES)),
                               trace=bool(_trace))
    outs = [np.asarray(res.results[c]["ypred"], dtype=np.float32)
            for c in range(NCORES)]
    full = np.concatenate(outs, axis=0)
    if _trace:
        return full, res
    return full
